# revision 43
# baseline (speedup 1.0000x reference)
"""MoE gate (group-limited top-k routing) as a Bass/Tile kernel for 8 TRN2 cores.

Computes, per token:
  logits = hidden @ W            (K=7168, E=256)
  scores = sigmoid(logits) + bias
  group-limited routing: top-2-sum per group of 32 -> top-4 groups of 8
  top-8 of masked scores, renormalized, * 2.5

Sharding: data-parallel over tokens (1024 tokens/core), W + bias replicated.

Matmul scheme (f16x1): quantize x and W to fp16 and do a single fp16 matmul
per K-chunk with fp32 PSUM accumulation. The logits error is ~3e-3 absolute,
and the output is only the renormalized top-8 *values*, so near-tie routing
flips cost ~nothing: measured routing L2 rel err ~1.8e-4 (gate 2e-2).

Key design points (each measured on HW):
- x split to fp16 on ACT *before* the PE transpose: transpose streams 1 pass
  (128 cycles) instead of 4; per K-chunk the PE does 128+256 fp16 streaming
  cycles vs 512+512+256 for the fp32-exact hi/lo scheme.
- K-chunk (s, j) = hidden rows {1024 s + 8 m + j}: this strided chunking
  lets the weight DMA merge 8 consecutive rows into 8KB-per-partition
  descriptors (vs 1KB for contiguous chunks), which keeps the mixed
  weight+hidden DMA stream at the full ~410 GB/s single-ring rate. The
  matching 16B-strided LDWEIGHTS costs only ~9ns/chunk.
- Matmuls LAG the transposes by 1-2 tiles (greedy drain): during the weight
  load the PE can never be filled by transposes alone (0.77us PE work per MB
  vs 2.44us/MB DMA), so weights spread 1-per-2-hidden-slices and each
  arriving weight batch unlocks matmul work for ALL transposed tiles.
- Dummy-matmul burst at start warms the PE HAM clock gate (else the whole
  ramp runs at 1.2 GHz instead of 2.4).

242.5us (fp32-exact baseline) -> ~134-136us measured; ~10us framework
preamble + ~90us DMA/PE-matched stream (36.7 MB/core at ~410 GB/s, PE ~86us
busy) + ~7us tail.
"""

import sys

if "/opt/trn_rl_repo" not in sys.path:
    sys.path.insert(0, "/opt/trn_rl_repo")

import numpy as np

import concourse.bacc as bacc
import concourse.bass as bass
import concourse.mybir as mybir
import concourse.tile as tile
from concourse import bass_utils
from concourse.masks import make_identity

P = 128
TOP_K = 8
N_GROUP = 8
TOPK_GROUP = 4
SCALE = 2.5

N_CORES = 8
TOKENS = 8192
HIDDEN = 7168
EXPERTS = 256


def build_moe_gate(
    tokens_per_core=TOKENS // N_CORES,
    hidden=HIDDEN,
    n_experts=EXPERTS,
):
    KC = hidden // P           # K-chunks of 128
    TT = tokens_per_core // P  # token tiles of 128
    GS = n_experts // N_GROUP  # experts per group
    DB = 8                     # K-chunks per DMA slice (512KB; dispatch-rate bound)
    ND = KC // DB              # DMA slices per token tile
    BATCH = 8                  # K-chunks per transpose/copyback batch
    NB = KC // BATCH           # processing batches per token tile
    f32 = mybir.dt.float32
    f16 = mybir.dt.float16

    nc = bacc.Bacc("TRN2", target_bir_lowering=False, debug=False)
    hs = nc.dram_tensor(
        "hidden_states", [tokens_per_core, hidden], f32, kind="ExternalInput"
    ).ap()
    wk = nc.dram_tensor("kernel", [hidden, n_experts], f32, kind="ExternalInput").ap()
    bias = nc.dram_tensor(
        "e_score_correction_bias", [n_experts], f32, kind="ExternalInput"
    ).ap()
    out = nc.dram_tensor(
        "topk_out", [tokens_per_core, TOP_K], f32, kind="ExternalOutput"
    ).ap()

    with tile.TileContext(nc) as tc:
        with (
            tc.tile_pool(name="const", bufs=1) as cpool,
            tc.tile_pool(name="wstage", bufs=4) as wspool,
            tc.tile_pool(name="hload", bufs=12) as hpool,
            tc.tile_pool(name="warm", bufs=1, space="PSUM") as warmpool,
            tc.tile_pool(name="hsplit", bufs=6) as hspool,
            tc.tile_pool(name="ht", bufs=24) as htpool,
            tc.tile_pool(name="ptr", bufs=3, space="PSUM") as ptpool,
            tc.tile_pool(name="plog", bufs=2, space="PSUM") as plpool,
            tc.tile_pool(name="route", bufs=2) as rpool,
        ):
            identity = cpool.tile([P, P], f16)
            make_identity(nc, identity)

            # HAM warm-up: ~5us of dummy matmuls while the PE would otherwise
            # idle waiting for the weight DMAs. Flips the PE clock gate to
            # 8/8 (2.4 GHz) before real work arrives; without this the whole
            # startup phase runs transposes/matmuls at 1.2 GHz.
            warm_ps = warmpool.tile([P, P], f32)
            for _ in range(48):
                nc.tensor.matmul(warm_ps, lhsT=identity, rhs=identity)

            # PROBE layout: global chunk (s, j) = rows {1024 s + 8 m + j};
            # weight DMA cost is identical (1KB rows either way), but the
            # transpose stationary reads become 16B-strided — measures
            # whether strided LDWEIGHTS is free
            whi = cpool.tile([P, ND, DB, n_experts], f16)
            wk_view = wk.rearrange("(s m j) e -> m s j e", m=P, j=DB)

            def load_weight_batch(wb):
                # fp32 stage on the shared sync ring; fp16 convert split
                # between DVE and the otherwise-idle gpsimd. The DVE runs
                # ~75% busy in the ramp and a full 1.2us cast in its
                # in-order queue delays the PSUM->SBUF copybacks that the
                # lagged matmuls wait on; gpsimd is ~6x slower per element
                # but runs in parallel and its half still lands within the
                # matmul drain margin. (All-ACT overloads the hi-splits.)
                wstage = wspool.tile([P, DB, n_experts], f32)
                nc.sync.dma_start(out=wstage, in_=wk_view[:, wb, :, :])
                h = DB // 2
                nc.vector.tensor_copy(whi[:, wb, :h, :], wstage[:, :h, :])
                nc.gpsimd.tensor_copy(whi[:, wb, h:, :], wstage[:, h:, :])

            # bias is only needed by the first routing epilogue, well into the
            # run
            bias_sb = cpool.tile([P, n_experts], f32)
            bias_bcast = bass.AP(
                tensor=bias.tensor, offset=bias.offset, ap=[[0, P]] + list(bias.ap)
            )
            nc.gpsimd.dma_start(out=bias_sb, in_=bias_bcast)

            # all 8 token tiles' outputs land here; one DMA at the end
            wout_all = cpool.tile([P, TT, TOP_K], f32)

            # ---- main pipeline ----
            # The PE can never be full during the weight load: transposes
            # alone deliver only ~0.77us of PE work per MB of hidden while
            # the DMA delivers 1 MB per ~2.44us, and matmuls are
            # weight-gated. So the matmuls LAG the transposes by up to two
            # tiles: each arriving weight batch then unlocks matmul work for
            # ALL already-transposed tiles, and the weights can spread
            # thinly through the stream (one batch per two hidden slices)
            # without starving the PE. Matmul schedule per tile:
            #   tiles 0,1: transposes only
            #   tile 2: M(0)   tile 3: M(1)   tile 4: M(2)+M(3) (catch-up)
            #   tile t>=5: M(t-1)   after tile 7: M(7)
            hiT = [[None] * NB for _ in range(TT)]
            logits = [None] * TT
            wb_loaded = [0]

            def do_transpose(t, b):
                # PE transposes for batch b (PSUM), then PSUM->SBUF copyback
                # on DVE in two halves so the first half's matmuls never
                # wait on the second. Chunk j of slice b = strided columns
                # {8 m + j}, matching the 8KB-descriptor weight layout.
                hi_v = hi_dma[t][b].rearrange("p (m j) -> p j m", j=DB)
                tp = ptpool.tile([P, BATCH * P], f16)
                for j in range(BATCH):
                    nc.tensor.transpose(
                        tp[:, j * P : (j + 1) * P], hi_v[:, j, :], identity
                    )
                ht = htpool.tile([P, BATCH * P], f16)
                half = BATCH * P // 2
                nc.vector.tensor_copy(ht[:, :half], tp[:, :half])
                nc.vector.tensor_copy(ht[:, half:], tp[:, half:])
                hiT[t][b] = ht

            def do_matmuls(t, b):
                if b == 0:
                    logits[t] = plpool.tile(
                        [P, n_experts], f32, name="logits_ps"
                    )
                for j in range(BATCH):
                    k = b * BATCH + j
                    nc.tensor.matmul(
                        logits[t],
                        lhsT=hiT[t][b][:, j * P : (j + 1) * P],
                        rhs=whi[:, b, j, :],
                        start=(k == 0),
                        stop=(k == KC - 1),
                    )

            def dma_tile_slice(t, s):
                sl = slice(s * DB * P, (s + 1) * DB * P)
                htile = hpool.tile([P, DB * P], f32)
                nc.sync.dma_start(out=htile, in_=hs[t * P : (t + 1) * P, sl])
                # one weight batch per two hidden slices until all loaded
                if (t * ND + s) % 2 == 0 and wb_loaded[0] < ND:
                    load_weight_batch(wb_loaded[0])
                    wb_loaded[0] += 1
                hi = hspool.tile([P, DB * P], f16)
                nc.scalar.activation(hi, htile, mybir.ActivationFunctionType.Copy)
                hi_dma[t].append(hi)

            def epilogue(t):
                # ---- routing epilogue for tile t (tokens on partitions) ----
                sc = rpool.tile([P, n_experts], f32)
                nc.scalar.activation(
                    sc, logits[t], mybir.ActivationFunctionType.Sigmoid
                )
                nc.vector.tensor_add(sc, sc, bias_sb)

                # top-2 sum per group of GS experts
                m8 = rpool.tile([P, N_GROUP * 8], f32)
                for g in range(N_GROUP):
                    nc.vector.max(
                        m8[:, g * 8 : (g + 1) * 8], sc[:, g * GS : (g + 1) * GS]
                    )
                m8v = m8.rearrange("p (g k) -> p g k", k=8)
                gsum = rpool.tile([P, N_GROUP], f32)
                nc.vector.tensor_add(gsum, m8v[:, :, 0], m8v[:, :, 1])

                # top-TOPK_GROUP groups -> per-group 0/1 mask via threshold
                gmax = rpool.tile([P, 8], f32)
                nc.vector.max(gmax, gsum)
                gmask = rpool.tile([P, N_GROUP], f32)
                nc.vector.tensor_scalar(
                    gmask,
                    gsum,
                    gmax[:, TOPK_GROUP - 1 : TOPK_GROUP],
                    None,
                    op0=mybir.AluOpType.is_ge,
                )

                # masked scores = sc * mask (0 where group dropped)
                masked = rpool.tile([P, n_experts], f32)
                nc.vector.tensor_mul(
                    masked.rearrange("p (g e) -> p g e", g=N_GROUP),
                    sc.rearrange("p (g e) -> p g e", g=N_GROUP),
                    gmask[:, :, None].broadcast_to([P, N_GROUP, GS]),
                )

                top8 = rpool.tile([P, TOP_K], f32)
                nc.vector.max(top8, masked)

                dsum = rpool.tile([P, 1], f32)
                nc.vector.reduce_sum(dsum, top8, axis=mybir.AxisListType.X)
                rcp = rpool.tile([P, 1], f32)
                nc.vector.reciprocal(rcp, dsum)
                nc.vector.tensor_scalar(
                    wout_all[:, t, :],
                    top8,
                    rcp,
                    SCALE,
                    op0=mybir.AluOpType.mult,
                    op1=mybir.AluOpType.mult,
                )

            # Greedy matmul drain: transposes emit eagerly; matmul (t', b')
            # is emitted once (a) tile t' is fully transposed... relaxed to
            # batch-transposed, (b) its weight batch b' has certainly landed
            # (weight b' rides after global hidden slice 2b'+1), with one
            # slice of margin. Tile 7 runs zero-lag so its matmuls overlap
            # its own transposes instead of trailing the DMA.
            hi_dma = [[] for _ in range(TT)]
            pending = []  # (t', b') transposed, matmuls not yet emitted
            drained = [0]

            def drain_matmuls(pos, keep=0):
                while drained[0] < len(pending) - keep:
                    tp_, bp_ = pending[drained[0]]
                    if 2 * bp_ + 2 > pos - 1:
                        break
                    do_matmuls(tp_, bp_)
                    drained[0] += 1
                    if bp_ == NB - 1:
                        epilogue(tp_)

            for t in range(TT):
                for b in range(NB):
                    dma_tile_slice(t, b)
                    do_transpose(t, b)
                    if t < TT - 1:
                        if t >= 1:
                            drain_matmuls(t * NB + b)
                        pending.append((t, b))
                    else:
                        # last tile: drain everything else, then run its own
                        # matmuls two batches behind its transposes so they
                        # never stall on a copyback
                        drain_matmuls(10 * NB, keep=0)
                        if b >= 2:
                            do_matmuls(TT - 1, b - 2)
                    # drip-feed tiny dummy matmuls through the early ramp so
                    # the PE activity monitor holds the clock at 2.4 GHz
                    if t < 2:
                        for _ in range(2):
                            nc.tensor.matmul(
                                warm_ps[:, :32], lhsT=identity, rhs=identity[:, :32]
                            )
            do_matmuls(TT - 1, NB - 2)
            do_matmuls(TT - 1, NB - 1)
            epilogue(TT - 1)

            nc.sync.dma_start(
                out=out.rearrange("(tt p) k -> p tt k", p=P), in_=wout_all
            )

    nc.compile()
    return nc


_CACHE = {}


def _built_nc():
    if "nc" not in _CACHE:
        _CACHE["nc"] = build_moe_gate()
    return _CACHE["nc"]


def kernel(hidden_states, kernel, e_score_correction_bias):
    hs = np.ascontiguousarray(np.asarray(hidden_states), dtype=np.float32)
    wk = np.ascontiguousarray(np.asarray(kernel), dtype=np.float32)
    bi = np.ascontiguousarray(np.asarray(e_score_correction_bias), dtype=np.float32)
    assert hs.shape == (TOKENS, HIDDEN) and wk.shape == (HIDDEN, EXPERTS)

    tpc = TOKENS // N_CORES
    nc = _built_nc()
    in_maps = [
        {
            "hidden_states": hs[i * tpc : (i + 1) * tpc],
            "kernel": wk,
            "e_score_correction_bias": bi,
        }
        for i in range(N_CORES)
    ]
    res = bass_utils.run_bass_kernel_spmd(nc, in_maps, core_ids=list(range(N_CORES)))
    return np.concatenate(
        [res.results[i]["topk_out"] for i in range(N_CORES)], axis=0
    )
